# revision 3
# baseline (speedup 1.0000x reference)
"""Trainium2 Bass kernel for nn_AttentionFlow (T=8192, J=1024, D=256, 8 cores).

Reference math:
  w_c, w_q, w_m = w[:D], w[D:2D], w[2D:]
  S[t,j] = ctx@w_c [t] + q@w_q [j] + (ctx*w_m) @ q.T     [T, J]
  A = softmax_j(S);  c2q = A @ q                          [T, D]
  b = max_j S;       h = b @ ctx                          [D]
  G = [ctx, c2q, ctx*c2q, ctx*h]                          [T, 4D]

Sharding: rows (t) split across 8 cores, 1024 rows each. Only h crosses
cores. Host-side prep (make_in_maps): qwq = q@w_q, plus bf16 layout
variants of the matmul operands (ctx_bf, ctx.T, (q*w_m).T, [q|1]) so the
kernel does no on-device transposes or casts.

Per-core structure (bf16 matmuls, f32 PSUM accumulation; no softmax
max-subtraction needed since |S| <= ~6):

  rendezvous: a fire-and-forget all-core ncfw AllReduce marks the program
           has_collectives, which makes the runtime launch all 8 cores in
           lockstep (collective-free NEFFs free-run with ms dispatch
           stagger). Nothing waits on its result; the kernel-tail drain's
           wait on its completion sem is stripped post-schedule.
  phase A (high priority, per 128-row t-tile): S = qwq-seed (K=1 matmul)
           + ctx @ Qm.T in PSUM; rowmax -> m; cwc = ctx @ w_c; b = m+cwc.
           Then h partial = sum_t b_t ctx_t accumulated into a packed
           [128, 2] PSUM tile (two chained accumulation groups).
  h exchange: XOR all-gather via remote_dma_broadcast — each core sends
           its packed partial to peer (tpb ^ delta), slot delta, spread
           over 4 SWDGE queues; overlaps phases B/C. Replaces the ncfw
           AllReduce path (~80us ncfw latency vs ~10us direct DMA).
  phase B (per 128-col j-chunk): S.T = Qm @ ctx.T; exp via ACT with qwq
           as the per-partition bias -> E.T already transposed.
  phase C (per t-tile): U = E.T-chunks.T @ [q | 1]; the ones column gives
           softmax denominators; c2q = U[:, :D]/U[:, D]; assemble
           [ctx, c2q, ctx*c2q], 3KB-row DMA out (alternating HWDGE FIFOs).
  raw tail (post-Tile, manual semaphores survive only here): gated wait
           for 7 peer arrivals, tree-reduce, h unpacked via two PE
           transposes + two K=1 ones-matmuls into a broadcast [128, 256]
           PSUM tile, G4 = ctx*h, two half-block output DMAs. All raw
           sems cleared at the end (device sems persist across runs).
"""

import sys

if "/opt/trn_rl_repo" not in sys.path:
    sys.path.insert(0, "/opt/trn_rl_repo")

import numpy as np

import concourse.bass as bass
import concourse.bacc as bacc
import concourse.tile as tile
from concourse import mybir
from concourse.bass_utils import run_bass_kernel_spmd
from concourse.masks import make_identity
from concourse.tile_rust import add_dep_helper

T, J, D = 8192, 1024, 256
N_CORES = 8
T_LOC = T // N_CORES          # 1024 rows per core
NT = T_LOC // 128             # 8 t-tiles per core
NJ = J // 128                 # 8 j-chunks
F32 = mybir.dt.float32
BF16 = mybir.dt.bfloat16


def _build_program():
    nc = bacc.Bacc("TRN2", target_bir_lowering=False, debug=False,
                   num_devices=N_CORES, num_swdge_queues=4)
    ctx_ap = nc.dram_tensor("context", [T_LOC, D], F32, kind="ExternalInput").ap()
    ctxbf_ap = nc.dram_tensor("ctx_bf", [T_LOC, D], BF16, kind="ExternalInput").ap()
    ctxT_ap = nc.dram_tensor("ctxT_bf", [D, T_LOC], BF16, kind="ExternalInput").ap()
    qmT_ap = nc.dram_tensor("qmT_bf", [D, J], BF16, kind="ExternalInput").ap()
    qaug_ap = nc.dram_tensor("qaug_bf", [J, D + 1], BF16, kind="ExternalInput").ap()
    w_ap = nc.dram_tensor("w", [3 * D], F32, kind="ExternalInput").ap()
    qwq_ap = nc.dram_tensor("qwq", [J], F32, kind="ExternalInput").ap()
    out_ap = nc.dram_tensor("out", [T_LOC, 4 * D], F32, kind="ExternalOutput").ap()
    warm_ap = nc.dram_tensor("warm", [128, 1], F32, kind="ExternalOutput").ap()
    dbg_ap = nc.dram_tensor("dbg", [128, 24], F32, kind="ExternalOutput").ap()

    # Raw (non-Tile) SBUF/DRAM tensors with concrete addresses: everything
    # the raw post-Tile tail touches must be non-symbolic. Allocated before
    # TileContext so Tile's allocator works above them.
    raw = dict(
        ctx_f32=nc.alloc_sbuf_tensor("rt_ctx", [128, NT, D], F32),
        rbuf=nc.alloc_sbuf_tensor("rt_rbuf", [128, 8, 2], F32),
        h_bc=nc.alloc_sbuf_tensor("rt_hbc", [128, D], F32),
        g4_all=nc.alloc_sbuf_tensor("rt_g4", [128, NT, D], F32),
        t1=nc.alloc_sbuf_tensor("rt_t1", [128, 4, 2], F32),
        t2=nc.alloc_sbuf_tensor("rt_t2", [128, 2, 2], F32),
        h_pack=nc.alloc_sbuf_tensor("rt_hpack", [128, 2], F32),
        ident_f32=nc.alloc_sbuf_tensor("rt_idf32", [128, 128], F32),
        ones_f32=nc.alloc_sbuf_tensor("rt_ones", [1, 128], F32),
        hrow=nc.alloc_sbuf_tensor("rt_hrow", [1, D], F32),
        h_lin=nc.dram_tensor("rt_hlin", [1, D], F32),
        out_ap=out_ap,
        dbg_ap=dbg_ap,
        # pre-allocated so they cannot collide with Tile-managed sems that
        # get freed back to the pool at TileContext exit
        hsem=nc.alloc_semaphore("rt_hsem"),
        vsem=nc.alloc_semaphore("rt_vsem"),
        gsem=nc.alloc_semaphore("rt_gsem"),
    )

    aps = dict(ctx=ctx_ap, ctxbf=ctxbf_ap, ctxT=ctxT_ap, qmT=qmT_ap,
               qaug=qaug_ap, w=w_ap, qwq=qwq_ap)
    with tile.TileContext(nc) as tc:
        _emit(tc, out_ap, aps, warm_ap, raw)
        tc._emit_exitstack.close()
    # Flag-only collectives marker: requests the runtime's coordinated
    # multi-core launch without emitting any CC instruction (a real CC op
    # floors the NEFF end at the CC software stream's completion).
    nc.has_collectives = True
    _emit_raw_tail(nc, raw)
    nc.compile()
    return nc


def _emit(tc, out_ap, aps, warm_ap, raw):
    from contextlib import ExitStack
    nc = tc.nc
    ctx_ap = aps["ctx"]
    w_ap = aps["w"]
    qwq_ap = aps["qwq"]
    AF = mybir.ActivationFunctionType
    ALU = mybir.AluOpType

    es = ExitStack()
    tc._emit_exitstack = es
    singles = es.enter_context(tc.tile_pool(name="singles", bufs=1))
    wk_sm = es.enter_context(tc.tile_pool(name="wk_sm", bufs=4))
    wk_g = es.enter_context(tc.tile_pool(name="wk_g", bufs=8))
    ps_S = es.enter_context(tc.tile_pool(name="ps_S", bufs=3, space="PSUM"))
    ps_TC = es.enter_context(tc.tile_pool(name="ps_TC", bufs=2, space="PSUM"))
    ps_U = es.enter_context(tc.tile_pool(name="ps_U", bufs=2, space="PSUM"))
    ps_h = es.enter_context(tc.tile_pool(name="ps_h", bufs=1, space="PSUM"))
    dram = es.enter_context(tc.tile_pool(name="dram", bufs=1, space="DRAM"))

    # ---------------- one-time prep ----------------
    # has_collectives is set manually in _build_program (flag-only): the
    # runtime's coordinated-launch path keys on the NEFF attribute, while
    # an actual CC op would tie the NEFF's completion to the slow CC-core
    # software stream (~110us floor: CC boot + barrier + data phase).

    # f32 identity + ones row for the raw-tail h unpack (PE transpose path)
    ident_f32 = raw["ident_f32"]
    make_identity(nc, ident_f32[:])
    nc.vector.memset(raw["ones_f32"][:], 1.0)

    # PE warm-up spin: dense dummy matmuls while the input DMAs run, so the
    # HAM clock gate releases (1.2 -> 2.4 GHz) before the real matmuls start.
    # The result is sunk to a tiny output so nothing dead-code-eliminates it.
    warm_src = singles.tile([128, 512], BF16)
    nc.vector.memset(warm_src, 0.001)
    wps = ps_S.tile([128, 512], F32, tag="S")
    for i in range(40):
        nc.tensor.matmul(wps, warm_src[:, 0:128], warm_src, start=True,
                         stop=True)
    warm_sb = singles.tile([128, 1], F32)
    nc.vector.reduce_max(warm_sb, wps, axis=mybir.AxisListType.X)
    nc.sync.dma_start(out=warm_ap, in_=warm_sb)

    # qwq in partition-major column form (phase B activation bias)
    qwqT = singles.tile([128, NJ], F32)
    nc.sync.dma_start(out=qwqT, in_=qwq_ap.rearrange("(c p) -> p c", p=128))
    # qwq row (bf16) for the K=1 PSUM seed matmul in phase A
    qwq_f32row = singles.tile([1, J], F32)
    nc.sync.dma_start(out=qwq_f32row, in_=qwq_ap.rearrange("(a d) -> a d", a=1))
    qwq_bf = singles.tile([1, J], BF16)
    nc.scalar.copy(qwq_bf, qwq_f32row)
    ones_bf = singles.tile([1, 128], BF16)
    nc.vector.memset(ones_bf, 1.0)
    # w_c in partition-major form for the cwc matmuls
    wc_pm = singles.tile([128, 2], F32)
    nc.sync.dma_start(out=wc_pm, in_=w_ap[0:D].rearrange("(c p) -> p c", p=128))
    wc_pm_bf = singles.tile([128, 2], BF16)
    nc.scalar.copy(wc_pm_bf, wc_pm)

    # host-prepped bf16 operands: q_aug = [q|1], QmT = (q*w_m).T,
    # ctxT = ctx.T, ctx_bf — all straight DMA loads, no transposes/casts.
    q_aug = singles.tile([128, NJ, D + 1], BF16)
    nc.sync.dma_start(out=q_aug,
                      in_=aps["qaug"].rearrange("(c p) d -> p c d", p=128))
    QmT = singles.tile([128, 2, J], BF16)
    nc.sync.dma_start(out=QmT,
                      in_=aps["qmT"].rearrange("(c p) j -> p c j", p=128))

    # persistent per-core state (ctx_f32 is a raw tensor — the tail reads it)
    ctx_f32 = raw["ctx_f32"]
    ctx_bf = singles.tile([128, NT, D], BF16)
    ctxT_all = singles.tile([128, 2, T_LOC], BF16)
    ET_all = singles.tile([128, NJ, T_LOC], BF16)
    b_all = singles.tile([128, NT], BF16)

    # ---------------- prologue: load ctx in all three layouts ----------------
    nc.sync.dma_start(out=ctxT_all,
                      in_=aps["ctxT"].rearrange("(c p) t -> p c t", p=128))
    nc.scalar.dma_start(out=ctx_bf,
                        in_=aps["ctxbf"].rearrange("(c p) d -> p c d", p=128))
    for t in range(NT):
        rows = slice(t * 128, (t + 1) * 128)
        nc.sync.dma_start(out=ctx_f32[:, t, :], in_=ctx_ap[rows, :])

    # ---------------- phase A: S row-maxes, b, h accumulation ----------------
    # h partial accumulated in packed [128, 2] layout (h[c*128+p] at [p, c])
    # so it can go straight into the cross-core exchange. High priority:
    # the sooner b/ph2 complete, the sooner the exchange's SWDGE
    # descriptor storm runs — before the phase-C output DMAs need the
    # SDMA engines.
    _hp_ctx = tc.high_priority()
    _hp_ctx.__enter__()
    ph2 = ps_h.tile([128, 2], F32)
    for t in range(NT):
        m = wk_sm.tile([128, 3], F32)
        for jh in range(2):
            ps = ps_S.tile([128, 512], F32, tag="S")
            nc.tensor.matmul(ps, ones_bf, qwq_bf[:, jh * 512:(jh + 1) * 512],
                             start=True, stop=False)
            for dc in range(2):
                nc.tensor.matmul(
                    ps, ctxT_all[:, dc, t * 128:(t + 1) * 128],
                    QmT[:, dc, jh * 512:(jh + 1) * 512],
                    start=False, stop=(dc == 1))
            nc.vector.reduce_max(m[:, jh:jh + 1], ps, axis=mybir.AxisListType.X)
        nc.vector.tensor_max(m[:, 0:1], m[:, 0:1], m[:, 1:2])

        ps_c = ps_TC.tile([128, 1], F32, tag="T")
        for dc in range(2):
            nc.tensor.matmul(ps_c, ctxT_all[:, dc, t * 128:(t + 1) * 128],
                             wc_pm_bf[:, dc:dc + 1],
                             start=(dc == 0), stop=(dc == 1))
        nc.vector.tensor_add(b_all[:, t:t + 1], m[:, 0:1], ps_c)

    # h partial: 2 accumulation groups (one per packed column), emitted
    # back-to-back after the t-loop and chained so the scheduler cannot
    # reorder a later accumulate before the group's start matmul.
    prev = None
    for dc in range(2):
        for t in range(NT):
            mm = nc.tensor.matmul(ph2[:, dc:dc + 1],
                                  ctx_bf[:, t, dc * 128:(dc + 1) * 128],
                                  b_all[:, t:t + 1],
                                  start=(t == 0), stop=(t == NT - 1),
                                  skip_group_check=True)
            if prev is not None:
                add_dep_helper(mm.ins, prev.ins, reason="h acc order")
            prev = mm
    _hp_ctx.__exit__(None, None, None)

    # ---------------- h exchange sends (direct core-to-core DMA) ----------
    # XOR all-gather: each core broadcasts its packed partial to peer
    # (own_tpb ^ delta) slot delta; slot 0 is the local copy. The sends fire
    # here (overlapping phases B/C); the gated receive + reduce + G4 happen
    # in the raw post-Tile tail (_emit_raw_tail) where manual semaphore
    # waits are preserved.
    rsem = nc.alloc_semaphore("h_rsem")
    lsem = nc.alloc_semaphore("h_lsem")
    rbuf = raw["rbuf"]
    with tc.high_priority():
        hp = singles.tile([128, 2], F32)
        nc.vector.tensor_copy(hp, ph2)
        nc.vector.tensor_copy(rbuf[:, 0, :], hp)
        # 7 sends spread over all 4 SWDGE queues so the descriptor rings
        # drain in parallel (a single ring serializes ~8us per broadcast).
        used_q = set()
        for delta in range(1, 8):
            rdests = [None] * 8
            rdests[delta] = (0, delta)
            q = delta % 4
            used_q.add(q)
            nc.gpsimd.remote_dma_broadcast(
                out_ap=rbuf[:, delta, :],
                in_ap=hp[:],
                remote_sem=rsem,
                local_sem=lsem,
                rdests=rdests,
                queue_num=q)
        for q in sorted(used_q):
            nc.gpsimd.trigger_dma(count=None, queue_num=q)
    raw["rsem"] = rsem
    raw["lsem"] = lsem

    # ---------------- phase B: E.T per j-chunk ----------------
    for jc in range(NJ):
        for th in range(2):
            ps = ps_S.tile([128, 512], F32, tag="S")
            for dc in range(2):
                nc.tensor.matmul(
                    ps, QmT[:, dc, jc * 128:(jc + 1) * 128],
                    ctxT_all[:, dc, th * 512:(th + 1) * 512],
                    start=(dc == 0), stop=(dc == 1))
            nc.scalar.activation(ET_all[:, jc, th * 512:(th + 1) * 512], ps,
                                 AF.Exp, bias=qwqT[:, jc:jc + 1])

    # ---------------- phase C: U, c2q, G[:, 0:768] per t-tile ----------------
    for t in range(NT):
        rows = slice(t * 128, (t + 1) * 128)
        pu = ps_U.tile([128, D + 1], F32, tag="U")
        for jc in range(NJ):
            nc.tensor.matmul(pu, ET_all[:, jc, t * 128:(t + 1) * 128],
                             q_aug[:, jc, :],
                             start=(jc == 0), stop=(jc == NJ - 1))
        r = wk_sm.tile([128, 1], F32, tag="recip")
        nc.vector.reciprocal(r, pu[:, D:D + 1])
        g123 = wk_g.tile([128, 3 * D], F32, tag="g123")
        nc.vector.tensor_copy(g123[:, 0:D], ctx_f32[:, t, :])
        nc.scalar.activation(g123[:, D:2 * D], pu[:, 0:D], AF.Copy, scale=r)
        nc.vector.tensor_mul(g123[:, 2 * D:3 * D], ctx_f32[:, t, :],
                             g123[:, D:2 * D])
        eng = nc.scalar if t % 2 == 0 else nc.sync
        eng.dma_start(out=out_ap[rows, 0:3 * D], in_=g123)

    # (phase D moved to the raw post-Tile tail)


def _emit_raw_tail(nc, st):
    """Raw-bass tail after TileContext: gated receive of the h partials,
    reduce, unpack to a broadcast row, G4 = ctx*h, output DMA. Raw emission
    keeps manual semaphore waits intact (Tile's sem assigner drops them) and
    per-engine program order. Sems are cleared at the end so reruns of the
    same NEFF start from zero."""
    rsem = st["rsem"]
    rbuf = st["rbuf"]
    h_lin = st["h_lin"]
    h_bc = st["h_bc"]
    ctx_f32 = st["ctx_f32"]
    g4_all = st["g4_all"]
    out_ap = st["out_ap"]

    t1 = st["t1"]
    t2 = st["t2"]
    h_pack = st["h_pack"]
    hsem = st["hsem"]
    vsem = st["vsem"]
    gsem = st["gsem"]

    # all 7 peers arrived (2 incs each). Raw same-engine chains need drains
    # between dependent DVE ops (no auto-interlock outside Tile).
    nc.vector.wait_ge(rsem, 14)
    nc.vector.tensor_add(t1[:], rbuf[:, 0:4, :], rbuf[:, 4:8, :])
    nc.vector.drain()
    nc.vector.tensor_add(t2[:], t1[:, 0:2, :], t1[:, 2:4, :])
    nc.vector.drain()
    nc.vector.tensor_add(h_pack[:], t2[:, 0, :], t2[:, 1, :]).then_inc(vsem)
    nc.vector.sem_clear(rsem)

    # unpack via PE (no DRAM round trip): two [128,1] transposes give h as
    # a [1, 256] row; two K=1 matmuls against a ones-column broadcast it
    # across all 128 partitions.
    ident_f32 = st["ident_f32"]
    ones_f32 = st["ones_f32"]
    hrow = st["hrow"]
    psT = nc.place_psum_tensor("rt_psT", [1, D], F32, bank=0)
    psB = nc.place_psum_tensor("rt_psB", [128, D], F32, bank=1)
    nc.tensor.wait_ge(vsem, 1)
    for c in range(2):
        nc.tensor.transpose(psT[0:1, c * 128:(c + 1) * 128],
                            h_pack[:, c:c + 1], ident_f32[:])
    nc.tensor.drain()
    nc.tensor.sem_inc(hsem, 1)
    nc.vector.wait_ge(hsem, 1)
    nc.vector.tensor_copy(hrow[:], psT[0:1, :]).then_inc(hsem, 1)
    nc.tensor.wait_ge(hsem, 2)
    for c in range(2):
        nc.tensor.matmul(psB[:, c * 128:(c + 1) * 128], ones_f32[:],
                         hrow[0:1, c * 128:(c + 1) * 128],
                         start=True, stop=True)
    nc.tensor.drain()
    nc.tensor.sem_inc(hsem, 1)

    # G4 = ctx * h (h read straight out of PSUM; no SBUF copy). Two halves
    # so the first output DMA overlaps the second half's multiplies.
    nc.vector.wait_ge(hsem, 3)
    H = NT // 2
    for half in range(2):
        for t in range(half * H, (half + 1) * H):
            nc.vector.tensor_mul(g4_all[:, t, :], ctx_f32[:, t, :], psB[:])
        nc.vector.drain()
        nc.vector.sem_inc(vsem, 1)
    nc.sync.wait_ge(vsem, 2)
    nc.sync.dma_start(
        out=out_ap[0:H * 128, 3 * D:4 * D].rearrange("(t p) d -> p t d", p=128),
        in_=g4_all[:, 0:H, :]).then_inc(gsem, 16)
    nc.sync.wait_ge(vsem, 3)
    nc.sync.dma_start(
        out=out_ap[H * 128:T_LOC, 3 * D:4 * D].rearrange("(t p) d -> p t d", p=128),
        in_=g4_all[:, H:NT, :]).then_inc(gsem, 16)
    nc.sync.wait_ge(gsem, 32)

    # Reset every raw sem so the next execution of this NEFF starts clean
    # (device semaphores persist across executions and even processes).
    # lsem is left uncleaned: nothing waits on it, and its increments can
    # arrive late (dummy broadcast lanes) — clearing would race them.
    nc.sync.sem_clear(hsem)
    nc.sync.sem_clear(vsem)
    nc.sync.sem_clear(gsem)


_NC_CACHE = None
_CLEANED = False


def _build_cleanup():
    """Tiny raw program that zeroes the bass kernel semaphore space
    (153..255) on every core. Device semaphores persist across NEFF
    executions and processes; a previously interrupted run would otherwise
    poison this kernel's manual cross-core semaphores."""
    nc = bacc.Bacc("TRN2", target_bir_lowering=False, debug=False,
                   num_devices=N_CORES)
    out_ap = nc.dram_tensor("clout", [128, 1], F32, kind="ExternalOutput").ap()
    buf = nc.alloc_sbuf_tensor("clbuf", [128, 1], F32)
    nc.gpsimd.dma_reset(range(153, 256))
    nc.gpsimd.sem_clear(range(153, 256))
    csem = nc.alloc_semaphore("clsem")
    nc.gpsimd.sem_inc(csem, 1)
    nc.vector.wait_ge(csem, 1)
    nc.vector.memset(buf[:], 1.0)
    nc.vector.sem_inc(csem, 1)
    nc.sync.wait_ge(csem, 2)
    nc.sync.dma_start(out=out_ap, in_=buf[:]).then_inc(csem, 16)
    nc.sync.wait_ge(csem, 18)
    nc.sync.sem_clear(csem)
    nc.compile()
    return nc


def _run_cleanup():
    global _CLEANED
    if _CLEANED:
        return
    _CLEANED = True
    nc = _build_cleanup()
    in_maps = [{} for _ in range(N_CORES)]
    run_bass_kernel_spmd(nc, in_maps, core_ids=list(range(N_CORES)))


def _get_program():
    global _NC_CACHE
    if _NC_CACHE is None:
        _NC_CACHE = _build_program()
    return _NC_CACHE


def make_in_maps(context: np.ndarray, query: np.ndarray, w: np.ndarray):
    """Host-side input prep: sharding plus layout/dtype transforms (qwq row,
    bf16 casts, transposes, [q|1] augmentation)."""
    import ml_dtypes
    bf16 = ml_dtypes.bfloat16
    context = np.ascontiguousarray(context, dtype=np.float32)
    query = np.ascontiguousarray(query, dtype=np.float32)
    w = np.ascontiguousarray(w, dtype=np.float32)
    qwq = query @ w[D:2 * D]
    qm = query * w[2 * D:]
    qmT = np.ascontiguousarray(qm.T.astype(bf16))
    qaug = np.concatenate(
        [query, np.ones((J, 1), dtype=np.float32)], axis=1).astype(bf16)
    in_maps = []
    for i in range(N_CORES):
        shard = context[i * T_LOC:(i + 1) * T_LOC]
        in_maps.append({
            "context": shard,
            "ctx_bf": shard.astype(bf16),
            "ctxT_bf": np.ascontiguousarray(shard.T.astype(bf16)),
            "qmT_bf": qmT,
            "qaug_bf": qaug,
            "w": w,
            "qwq": qwq,
        })
    return in_maps


def kernel(context: np.ndarray, query: np.ndarray, w: np.ndarray,
           **kwargs) -> np.ndarray:
    _run_cleanup()
    nc = _get_program()
    in_maps = make_in_maps(context, query, w)
    res = run_bass_kernel_spmd(nc, in_maps, core_ids=list(range(N_CORES)))
    return np.concatenate([res.results[i]["out"] for i in range(N_CORES)],
                          axis=0)

